# revision 33
# baseline (speedup 1.0000x reference)
"""Trainium2 Bass kernel for a 3-layer dense transformer (BigramModel).

Contract: kernel(**inputs) takes the FULL unsharded numpy inputs (as produced
by setup_inputs) and returns the full [B, T, V] float32 logits. Internally the
batch dim B=128 is sharded 16-per-core across 8 NeuronCores (pure data
parallelism, weights replicated), one Bass/Tile NEFF run via
run_bass_kernel_spmd.

v2 design notes (vs the v1 baseline that was Sync-engine bound at 2.6ms):
  - All XBAR DMA transposes are batched 12-into-1: one dma_start_transpose per
    512-token block turns [128, 4, 384] token-major into [128, 12, 128]
    feature-major (c12 = j*3 + c). 1536 transpose instructions -> ~80.
  - Layers run as two passes (attention pass over all 8 blocks, then MLP
    pass) so LN stats + rstd are hoisted: rstd = reciprocal(sqrt(var+eps))
    costs one ACT Sqrt (table switch) + one DVE reciprocal per pass instead
    of Ln/Exp table thrash per block (real HW puts Ln and Exp in different
    ACT table sets; v1 paid 112 x 1.3us table loads).
  - proj and W2 run token-major (lhsT = transposed activations, rhs = W) so
    the residual add is one scalar_tensor_tensor from PSUM into h -- no
    reverse transposes, no separate bias/copy ops.
  - attention o accumulates both key-halves in one PSUM bank (start/stop),
    evacuated by per-head DVE reciprocal + tensor_scalar (no ACT copies).
  - softmax exp stays on ACT; W1 relu evac alternates ACT/DVE to balance.
"""

import numpy as np
import ml_dtypes

BF16 = ml_dtypes.bfloat16

P = 128
T = 256
E = 384
V = 65
H = 6
HS = 64
FF = 1536
L = 3
NCORES = 8
BPC = 16              # sequences per core
TOK = BPC * T         # 4096 tokens per core
NT = TOK // P         # 32 token tiles
NB = TOK // 512       # 8 blocks of 512 tokens (2 seqs)
ECH = E // P          # 3
FCH = FF // P         # 12

_NC_CACHE = {}
PIPE = True


def _build_nc(flags):
    """Build + compile the Bass program.

    flags = (bq_nz, bk_nz, bv_nz, bp_nz, b1_nz, b2_nz, bout_nz) with per-layer
    tuples for the first six."""
    import concourse.bacc as bacc
    import concourse.mybir as mybir
    import concourse.tile as tile

    dt = mybir.dt
    f32 = dt.float32
    bf = dt.bfloat16
    Alu = mybir.AluOpType
    Act = mybir.ActivationFunctionType

    nc = bacc.Bacc("TRN2", target_bir_lowering=False, debug=False, num_devices=1)

    bq_nz, bk_nz, bv_nz, bp_nz, b1_nz, b2_nz, bout_nz = flags

    # ---- DRAM tensors ----
    D = {}
    D["oh"] = nc.dram_tensor("oh", [P, TOK], bf, kind="ExternalInput")
    D["te"] = nc.dram_tensor("te", [V, E], bf, kind="ExternalInput")
    D["pos"] = nc.dram_tensor("pos", [P, 2, E], f32, kind="ExternalInput")
    D["mask"] = nc.dram_tensor("mask", [P, P], bf, kind="ExternalInput")
    for l in range(L):
        for w in ("wq", "wk", "wv", "wproj"):
            D[f"{w}{l}"] = nc.dram_tensor(f"{w}{l}", [P, ECH, E], bf, kind="ExternalInput")
        D[f"w1{l}"] = nc.dram_tensor(f"w1{l}", [P, ECH, FF], bf, kind="ExternalInput")
        D[f"w2{l}"] = nc.dram_tensor(f"w2{l}", [P, FCH, E], bf, kind="ExternalInput")
        if bq_nz[l]:
            D[f"bq{l}"] = nc.dram_tensor(f"bq{l}", [P, ECH], f32, kind="ExternalInput")
        if bk_nz[l]:
            D[f"bk{l}"] = nc.dram_tensor(f"bk{l}", [P, ECH], f32, kind="ExternalInput")
        if bv_nz[l]:
            D[f"bvrow{l}"] = nc.dram_tensor(f"bvrow{l}", [1, E], bf, kind="ExternalInput")
        if bp_nz[l]:
            D[f"bprow{l}"] = nc.dram_tensor(f"bprow{l}", [1, E], bf, kind="ExternalInput")
        if b1_nz[l]:
            D[f"b1c{l}"] = nc.dram_tensor(f"b1c{l}", [P, FCH], f32, kind="ExternalInput")
        if b2_nz[l]:
            D[f"b2row{l}"] = nc.dram_tensor(f"b2row{l}", [1, E], bf, kind="ExternalInput")
    D["wout"] = nc.dram_tensor("wout", [P, ECH, V], bf, kind="ExternalInput")
    if bout_nz:
        D["boutc"] = nc.dram_tensor("boutc", [V, 1], f32, kind="ExternalInput")
    D["logT"] = nc.dram_tensor("logT", [V, TOK], f32, kind="ExternalOutput")

    with tile.TileContext(nc) as tc:
        import contextlib

        with contextlib.ExitStack() as ctx:
            const = ctx.enter_context(tc.tile_pool(name="const", bufs=1))
            wpool = ctx.enter_context(tc.tile_pool(name="wpool", bufs=2))
            act = ctx.enter_context(tc.tile_pool(name="act", bufs=2))
            acts = ctx.enter_context(tc.tile_pool(name="acts", bufs=3))
            act2 = ctx.enter_context(tc.tile_pool(name="act2", bufs=2))
            act1 = ctx.enter_context(tc.tile_pool(name="act1", bufs=2))
            ps_lin = ctx.enter_context(tc.tile_pool(name="ps_lin", bufs=4, space="PSUM"))
            ps_att = ctx.enter_context(tc.tile_pool(name="ps_att", bufs=4, space="PSUM"))

            # ---- constants ----
            # the K=65 embedding contraction is padded to K=128 host-side
            # (sub-128 partition matmuls are flaky on HW); pad rows are zero.
            te_sb = const.tile([P, E], bf, tag="te")
            nc.vector.memset(te_sb[:], 0.0)
            nc.sync.dma_start(out=te_sb[0:V, :], in_=D["te"].ap())
            pos_sb = const.tile([P, 2, E], f32, tag="pos")
            nc.sync.dma_start(out=pos_sb[:], in_=D["pos"].ap())
            mask_sb = const.tile([P, P], bf, tag="mask")
            nc.sync.dma_start(out=mask_sb[:], in_=D["mask"].ap())
            boutc_sb = None
            if bout_nz:
                boutc_sb = const.tile([V, 1], f32, tag="boutc")
                nc.sync.dma_start(out=boutc_sb[:], in_=D["boutc"].ap())
            ones_sb = const.tile([1, P], bf, tag="ones")
            nc.vector.memset(ones_sb[:], 1.0)
            eps_sb = const.tile([P, 1], f32, tag="eps")
            nc.vector.memset(eps_sb[:], 1e-5)
            zero_sb = const.tile([P, 1], f32, tag="zero")
            nc.vector.memset(zero_sb[:], 0.0)

            # persistent residual tiles (token-major fp32)
            h = [const.tile([P, E], f32, tag=f"h{i}", name=f"h{i}") for i in range(NT)]

            # ---- LN stats machinery (pipelined: sum(h) rides the accum_out
            # of each tile's LAST residual-update STT inside the previous
            # pass; only sum(h^2) costs an extra DVE op per tile; the cheap
            # finish step runs at pass start) ----

            def new_stats(tag):
                """Allocate (S, SS) accumulators for one LN pass."""
                s = const.tile([P, NT, 2], f32, tag=f"sv_{tag}", name=f"sv_{tag}")
                return s

            def emit_stats(sv, i0, n=4):
                """sum(h^2) for tiles i0..i0+n-1 (sum(h) already accumulated)."""
                for i in range(i0, i0 + n):
                    dm = acts.tile([P, E], f32, tag="stat_dm")
                    nc.vector.scalar_tensor_tensor(
                        out=dm[:], in0=h[i][:], scalar=0.0, in1=h[i][:],
                        op0=Alu.add, op1=Alu.mult, accum_out=sv[:, i, 1:2],
                    )

            def finish_range(sv, lo, hi, tag, fin):
                """(S, SS)[lo:hi] -> (m, rstd) tiles appended to fin.

                Split finishes let the next pass's early blocks start before
                the current pass's last block lands (independent tiles)."""
                n = hi - lo
                m = const.tile([P, n], f32, tag=f"m_{tag}_{lo}", name=f"m_{tag}_{lo}")
                nc.vector.tensor_scalar_mul(
                    out=m[:], in0=sv[:, lo:hi, 0], scalar1=1.0 / E)
                var = acts.tile([P, NT], f32, tag="var")
                nc.vector.scalar_tensor_tensor(
                    out=var[:, 0:n], in0=m[:], scalar=0.0, in1=m[:],
                    op0=Alu.add, op1=Alu.mult,
                )
                nc.vector.scalar_tensor_tensor(
                    out=var[:, 0:n], in0=sv[:, lo:hi, 1], scalar=1.0 / E,
                    in1=var[:, 0:n], op0=Alu.mult, op1=Alu.subtract,
                )
                sd = acts.tile([P, NT], f32, tag="sd")
                nc.scalar.activation(
                    out=sd[:, 0:n], in_=var[:, 0:n], func=Act.Sqrt, bias=eps_sb[:],
                )
                rstd = const.tile([P, n], f32, tag=f"rs_{tag}_{lo}",
                                  name=f"rs_{tag}_{lo}")
                nc.vector.reciprocal(out=rstd[:], in_=sd[:, 0:n])
                fin.append((lo, hi, m, rstd))
                return fin

            def load_w(name, shape, dtp, tag=None):
                t = wpool.tile(shape, dtp, tag=tag or name[:-1])
                nc.sync.dma_start(out=t[:], in_=D[name].ap())
                return t

            def load_layer(l):
                W = {}
                W["wq"] = load_w(f"wq{l}", [P, ECH, E], bf)
                W["wk"] = load_w(f"wk{l}", [P, ECH, E], bf)
                W["wv"] = load_w(f"wv{l}", [P, ECH, E], bf)
                W["wproj"] = load_w(f"wproj{l}", [P, ECH, E], bf)
                W["w1"] = load_w(f"w1{l}", [P, ECH, FF], bf)
                W["w2"] = load_w(f"w2{l}", [P, FCH, E], bf)
                W["bq"] = load_w(f"bq{l}", [P, ECH], f32) if bq_nz[l] else None
                W["bk"] = load_w(f"bk{l}", [P, ECH], f32) if bk_nz[l] else None
                W["bvrow"] = load_w(f"bvrow{l}", [1, E], bf) if bv_nz[l] else None
                W["bprow"] = load_w(f"bprow{l}", [1, E], bf) if bp_nz[l] else None
                W["b1c"] = load_w(f"b1c{l}", [P, FCH], f32) if b1_nz[l] else None
                W["b2row"] = load_w(f"b2row{l}", [1, E], bf) if b2_nz[l] else None
                return W

            # layer-0 weights + wout go on the DMA queue before everything
            # else so the first attention pass never waits on them.
            Wcur = load_layer(0)
            wout = wpool.tile([P, ECH, V], bf, tag="wout")
            nc.sync.dma_start(out=wout[:], in_=D["wout"].ap())

            # ---- embedding: h = onehot.T @ tok_emb + pos ----
            sv1 = new_stats("l0a")
            fin1 = []
            for blk in range(NB):
                ohc = act2.tile([P, 512], bf, tag="ohc")
                nc.sync.dma_start(
                    out=ohc[:], in_=D["oh"].ap()[:, blk * 512:(blk + 1) * 512])
                for jj in range(4):
                    i = 4 * blk + jj
                    ps = ps_lin.tile([P, 512], f32, tag="mm")
                    nc.tensor.matmul(
                        ps[:, 0:E], ohc[:, jj * P:(jj + 1) * P], te_sb[:],
                        start=True, stop=True,
                    )
                    nc.vector.scalar_tensor_tensor(
                        out=h[i][:], in0=ps[:, 0:E], scalar=0.0,
                        in1=pos_sb[:, i % 2, :], op0=Alu.add, op1=Alu.add,
                        accum_out=sv1[:, i, 0:1],
                    )
                emit_stats(sv1, 4 * blk)
                if blk == NB - 2:
                    finish_range(sv1, 0, NT - 4, "l0a", fin1)
                elif blk == NB - 1:
                    finish_range(sv1, NT - 4, NT, "l0a", fin1)

            def fin_get(fin, i):
                for lo, hi, m, rstd in fin:
                    if lo <= i < hi:
                        return m, rstd, lo
                raise KeyError(i)

            def make_xnT(i0, fin):
                """xn = (h - m) * rstd for 4 tiles -> single batched transpose
                to feature-major [P, 12, 128] (c12 = j*3 + c)."""
                m, rstd, lo = fin_get(fin, i0)
                xn4 = act.tile([P, 4, E], bf, tag="xn4")
                for j in range(4):
                    k = i0 + j - lo
                    nc.vector.tensor_scalar(
                        out=xn4[:, j, :], in0=h[i0 + j][:],
                        scalar1=m[:, k:k + 1],
                        scalar2=rstd[:, k:k + 1],
                        op0=Alu.subtract, op1=Alu.mult,
                    )
                xnT = act.tile([P, 12, P], bf, tag="xnT")
                nc.sync.dma_start_transpose(
                    xnT[:], xn4[:].rearrange("p a b -> p (a b)"))
                return xnT

            def lin_fmaj(xnT, w_sb, bias_col, fch, tag, evac, pool=None):
                """feature-major out [P, fch, 512] bf16; evac in {dve, act, mixN}."""
                o = (pool or act).tile([P, fch, 512], bf, tag=tag, name=tag)
                rhs_view = xnT[:].rearrange("p (j c) a -> p c j a", c=ECH)
                for f in range(fch):
                    ps = ps_lin.tile([P, 512], f32, tag="mm")
                    for c in range(ECH):
                        nc.tensor.matmul(
                            ps[:], w_sb[:, c, f * P:(f + 1) * P], rhs_view[:, c],
                            start=(c == 0), stop=(c == ECH - 1),
                        )
                    use_act = (evac == "act") or (evac == "mix" and f % 2 == 0)
                    if use_act:
                        if bias_col is not None:
                            nc.scalar.activation(
                                out=o[:, f, :], in_=ps[:], func=Act.Copy,
                                bias=bias_col[:, f:f + 1])
                        else:
                            nc.scalar.copy(out=o[:, f, :], in_=ps[:])
                    else:
                        if bias_col is not None:
                            nc.vector.tensor_scalar_add(
                                out=o[:, f, :], in0=ps[:],
                                scalar1=bias_col[:, f:f + 1])
                        else:
                            nc.vector.tensor_copy(out=o[:, f, :], in_=ps[:])
                return o

            def lin_fmaj_relu(xnT, w_sb, bias_col, tag):
                """W1 + relu, evac alternating ACT/DVE."""
                o = act1.tile([P, FCH, 512], bf, tag=tag, name=tag)
                rhs_view = xnT[:].rearrange("p (j c) a -> p c j a", c=ECH)
                for f in range(FCH):
                    ps = ps_lin.tile([P, 512], f32, tag="mm")
                    for c in range(ECH):
                        nc.tensor.matmul(
                            ps[:], w_sb[:, c, f * P:(f + 1) * P], rhs_view[:, c],
                            start=(c == 0), stop=(c == ECH - 1),
                        )
                    if f % 2 == 0:
                        nc.scalar.activation(
                            out=o[:, f, :], in_=ps[:], func=Act.Relu,
                            bias=(bias_col[:, f:f + 1] if bias_col is not None else 0.0))
                    else:
                        if bias_col is not None:
                            nc.vector.tensor_scalar(
                                out=o[:, f, :], in0=ps[:],
                                scalar1=bias_col[:, f:f + 1], scalar2=zero_sb[:],
                                op0=Alu.add, op1=Alu.max,
                            )
                        else:
                            nc.vector.tensor_scalar_max(
                                out=o[:, f, :], in0=ps[:], scalar1=zero_sb[:],
                            )
                return o

            def lin_tmaj_resid(xT, w_sb, nch, brow, i0, sv=None):
                """h[i0+j] += xT_j @ W + brow, token-major: one STT per tile.
                If sv is given, the STT also accumulates sum(h_new) into
                sv[:, i, 0:1] (the next LN pass's S statistic)."""
                for j in range(4):
                    ps = ps_lin.tile([P, 512], f32, tag="mm")
                    for c in range(nch):
                        nc.tensor.matmul(
                            ps[:, 0:E], xT[:, j * nch + c, :] if nch == ECH
                            else xT[:, c, j * P:(j + 1) * P],
                            w_sb[:, c, :],
                            start=(c == 0),
                            stop=(c == nch - 1 and brow is None),
                        )
                    if brow is not None:
                        nc.tensor.matmul(
                            ps[:, 0:E], ones_sb[:], brow[:], start=False, stop=True,
                        )
                    nc.vector.scalar_tensor_tensor(
                        out=h[i0 + j][:], in0=ps[:, 0:E], scalar=0.0,
                        in1=h[i0 + j][:], op0=Alu.add, op1=Alu.add,
                        accum_out=(None if sv is None else sv[:, i0 + j, 0:1]),
                    )

            scale = float(HS) ** -0.5

            # ---- transformer layers ----
            for l in range(L):
                W = Wcur
                wq, wk, wv, wproj = W["wq"], W["wk"], W["wv"], W["wproj"]
                w1, w2 = W["w1"], W["w2"]
                bq, bk, bvrow = W["bq"], W["bk"], W["bvrow"]
                bprow, b1c, b2row = W["bprow"], W["b1c"], W["b2row"]

                # ======== attention pass ========
                sv2 = new_stats(f"m{l}")
                fin2 = []

                def att_tail(p, sv2=sv2, fin2=fin2, wproj=wproj, bprow=bprow, l=l):
                    lin_tmaj_resid(p[1], wproj, ECH, bprow, p[0], sv2)
                    emit_stats(sv2, p[0])
                    if p[0] == NT - 8:
                        finish_range(sv2, 0, NT - 4, f"m{l}", fin2)
                    elif p[0] == NT - 4:
                        finish_range(sv2, NT - 4, NT, f"m{l}", fin2)

                pend = None   # deferred proj of the previous block
                for b in range(NB):
                    i0 = 4 * b
                    xnT = make_xnT(i0, fin1)
                    QT = lin_fmaj(xnT, wq, bq, ECH, "QT", "dve")
                    KT = lin_fmaj(xnT, wk, bk, ECH, "KT", "act")
                    # V token-major, ones-augmented: [P, 4, H, 65]
                    Vt = act.tile([P, 4, H, 65], bf, tag="Vt")
                    for j in range(4):
                        ps = ps_lin.tile([P, 512], f32, tag="mm")
                        for c in range(ECH):
                            nc.tensor.matmul(
                                ps[:, 0:E], xnT[:, j * ECH + c, :], wv[:, c, :],
                                start=(c == 0),
                                stop=(c == ECH - 1 and bvrow is None),
                            )
                        if bvrow is not None:
                            nc.tensor.matmul(
                                ps[:, 0:E], ones_sb[:], bvrow[:],
                                start=False, stop=True,
                            )
                        nc.vector.tensor_copy(
                            out=Vt[:, j, :, 0:64],
                            in_=ps[:, 0:E].rearrange("p (h d) -> p h d", h=H),
                        )
                        nc.vector.memset(Vt[:, j, :, 64:65], 1.0)

                    # scores + exp + mask for BOTH sequences first (dense PE
                    # stream; exp latency hidden by the deferred proj below).
                    # scores are packed 2 heads (st=0) / 4 heads (st=1) per
                    # PSUM bank so each exp is one wide ACT instruction.
                    probs2 = []
                    for s in range(2):
                        tb = s * 256
                        probs = act2.tile([P, 2, H, 256], bf, tag="probs")
                        def score_mm(sc_reg, hh, st):
                            tlo = 128 if st == 1 else 0
                            c, off = divmod(hh * HS, P)
                            nc.tensor.matmul(
                                sc_reg,
                                KT[off:off + HS, c, tb + st * P: tb + (st + 1) * P],
                                QT[off:off + HS, c, tb + tlo: tb + 256],
                                start=True, stop=True,
                            )
                        for st in range(2):
                            tlo = 128 if st == 1 else 0
                            for hh in range(H):
                                sc = ps_att.tile([P, 512], f32, tag="att", name="sc")
                                score_mm(sc[:, 0:256 - tlo], hh, st)
                                nc.scalar.activation(
                                    out=probs[:, st, hh, tlo:256],
                                    in_=sc[:, 0:256 - tlo],
                                    func=Act.Exp, scale=scale,
                                )
                        # causal mask: only the diagonal 128x128 needs it
                        for st in range(2):
                            tlo = 128 if st == 1 else 0
                            nc.vector.tensor_tensor(
                                out=probs[:, st, :, tlo:tlo + P],
                                in0=probs[:, st, :, tlo:tlo + P],
                                in1=mask_sb[:, None, :].to_broadcast((P, H, P)),
                                op=Alu.mult,
                            )
                        probs2.append(probs)

                    # deferred proj of block b-1 fills the PE while exp runs
                    if PIPE and pend is not None:
                        att_tail(pend)

                    # o matmuls packed 4+2 heads per PSUM bank (all K=128,
                    # strict-FIFO drains -> no same-bank write races); softmax
                    # divide is one reciprocal + one broadcast TT per bank.
                    onorm4 = act.tile([P, 4, E], bf, tag="onorm4")
                    for s in range(2):
                        probs = probs2[s]
                        for tt in range(2):  # query tiles of this seq
                            for hg, nh in ((0, 4), (4, 2)):
                                po = ps_att.tile([P, 512], f32, tag="att", name="po")
                                for k in range(nh):
                                    hh = hg + k
                                    dst = po[:, k * P:k * P + 65]
                                    if tt == 0:
                                        nc.tensor.matmul(
                                            dst, probs[:, 0, hh, 0:P],
                                            Vt[:, 2 * s, hh, :],
                                            start=True, stop=True,
                                        )
                                    else:
                                        nc.tensor.matmul(
                                            dst, probs[:, 0, hh, P:256],
                                            Vt[:, 2 * s, hh, :],
                                            start=True, stop=False,
                                        )
                                        nc.tensor.matmul(
                                            dst, probs[:, 1, hh, P:256],
                                            Vt[:, 2 * s + 1, hh, :],
                                            start=False, stop=True,
                                        )
                                pv = po[:].rearrange("p (k c) -> p k c", c=P)
                                rec = acts.tile([P, 4], f32, tag="rec")
                                nc.vector.reciprocal(
                                    out=rec[:, 0:nh], in_=pv[:, 0:nh, 64:65])
                                nc.vector.tensor_tensor(
                                    out=onorm4[:, 2 * s + tt, hg * 64:(hg + nh) * 64]
                                    .rearrange("p (k d) -> p k d", d=64),
                                    in0=pv[:, 0:nh, 0:64],
                                    in1=rec[:, 0:nh, None].to_broadcast((P, nh, 64)),
                                    op=Alu.mult,
                                )
                    oT = act.tile([P, 12, P], bf, tag="oT")
                    nc.sync.dma_start_transpose(
                        oT[:], onorm4[:].rearrange("p a b -> p (a b)"))
                    if PIPE:
                        pend = (i0, oT)
                    else:
                        att_tail((i0, oT))
                if PIPE:
                    att_tail(pend)

                # ======== MLP pass (layer L-1 fuses the final LN+unembed
                # per block so the PE never drains at the kernel tail) ====
                fin2_c = fin2
                svn = new_stats("f" if l == L - 1 else f"a{l + 1}")
                finn = []
                last = l == L - 1

                def emit_unembed(i0b, finn=finn):
                    xnTf = make_xnT(i0b, finn)
                    rhs_view = xnTf[:].rearrange("p (j c) a -> p c j a", c=ECH)
                    ps = ps_lin.tile([V, 512], f32, tag="mm", name="mmv")
                    for c in range(ECH):
                        nc.tensor.matmul(
                            ps[:], wout[:, c, :], rhs_view[:, c],
                            start=(c == 0), stop=(c == ECH - 1),
                        )
                    lt = act2.tile([V, 512], f32, tag="lt")
                    if boutc_sb is not None:
                        nc.vector.tensor_scalar_add(
                            out=lt[:], in0=ps[:], scalar1=boutc_sb[:])
                    else:
                        nc.vector.tensor_copy(out=lt[:], in_=ps[:])
                    bb = i0b // 4
                    nc.sync.dma_start(
                        out=D["logT"].ap()[:, bb * 512:(bb + 1) * 512], in_=lt[:],
                    )

                def mlp_tail(p, svn=svn, finn=finn, w2=w2, b2row=b2row,
                             last=last, l=l):
                    lin_tmaj_resid(p[1], w2, FCH, b2row, p[0], svn)
                    emit_stats(svn, p[0])
                    if last:
                        finish_range(svn, p[0], p[0] + 4, "f", finn)
                        emit_unembed(p[0])
                    else:
                        if p[0] == NT - 8:
                            finish_range(svn, 0, NT - 4, f"a{l + 1}", finn)
                        elif p[0] == NT - 4:
                            finish_range(svn, NT - 4, NT, f"a{l + 1}", finn)

                pend = None   # deferred W2 of the previous block
                for b in range(NB):
                    i0 = 4 * b
                    xnT = make_xnT(i0, fin2_c)
                    aT = lin_fmaj_relu(xnT, w1, b1c, "aT")
                    if b == 1 and l + 1 < L:
                        # prefetch next layer's weights behind this pass
                        Wcur = load_layer(l + 1)
                    if PIPE and pend is not None:
                        mlp_tail(pend)
                    if PIPE:
                        pend = (i0, aT)
                    else:
                        mlp_tail((i0, aT))
                if PIPE:
                    mlp_tail(pend)
                fin1 = finn

    nc.compile()
    return nc


def _prep_shared(inp):
    """Host-side weight prep: layout rearrangement + LN gamma/beta folding."""
    sh = {}

    def f32(x):
        return np.asarray(x, np.float32)

    sh["te"] = np.asarray(f32(inp["tok_emb"]), BF16)                      # [V,E]
    sh["pos"] = np.ascontiguousarray(
        f32(inp["pos_emb"]).reshape(2, P, E).transpose(1, 0, 2))          # [P,2,E]
    sh["mask"] = np.asarray(np.triu(np.ones((P, P), np.float32)), BF16)   # [P,P]

    def tile3(w, fdim):  # [E, fdim] -> [P, ECH, fdim]
        return np.ascontiguousarray(w.reshape(ECH, P, fdim).transpose(1, 0, 2))

    def col(b, nch):  # [nch*P] -> [P, nch]
        return np.ascontiguousarray(b.reshape(nch, P).T)

    bq_nz, bk_nz, bv_nz, bp_nz, b1_nz, b2_nz = [], [], [], [], [], []
    for l in range(L):
        g1, b1_ = f32(inp["ln1_g"][l]), f32(inp["ln1_b"][l])
        g2, b2_ = f32(inp["ln2_g"][l]), f32(inp["ln2_b"][l])
        wq = f32(inp["Wq"][l]).transpose(1, 0, 2).reshape(E, E)   # head-major cols
        wk = f32(inp["Wk"][l]).transpose(1, 0, 2).reshape(E, E)
        wv = f32(inp["Wv"][l]).transpose(1, 0, 2).reshape(E, E)
        sh[f"wq{l}"] = np.asarray(tile3(g1[:, None] * wq, E), BF16)
        sh[f"wk{l}"] = np.asarray(tile3(g1[:, None] * wk, E), BF16)
        sh[f"wv{l}"] = np.asarray(tile3(g1[:, None] * wv, E), BF16)
        bq = wq.T @ b1_
        bk = wk.T @ b1_
        bv = wv.T @ b1_
        bq_nz.append(bool(np.any(bq != 0)))
        bk_nz.append(bool(np.any(bk != 0)))
        bv_nz.append(bool(np.any(bv != 0)))
        if bq_nz[-1]:
            sh[f"bq{l}"] = col(bq, ECH)
        if bk_nz[-1]:
            sh[f"bk{l}"] = col(bk, ECH)
        if bv_nz[-1]:
            sh[f"bvrow{l}"] = np.asarray(bv[None, :], BF16)
        wp = f32(inp["Wproj"][l])
        sh[f"wproj{l}"] = np.asarray(tile3(wp, E), BF16)
        bp = f32(inp["bproj"][l])
        bp_nz.append(bool(np.any(bp != 0)))
        if bp_nz[-1]:
            sh[f"bprow{l}"] = np.asarray(bp[None, :], BF16)
        w1 = f32(inp["W1"][l])
        sh[f"w1{l}"] = np.asarray(tile3(g2[:, None] * w1, FF), BF16)
        b1ff = f32(inp["b1"][l]) + w1.T @ b2_
        b1_nz.append(bool(np.any(b1ff != 0)))
        if b1_nz[-1]:
            sh[f"b1c{l}"] = col(b1ff, FCH)
        w2 = f32(inp["W2"][l])
        sh[f"w2{l}"] = np.asarray(
            w2.reshape(FCH, P, E).transpose(1, 0, 2), BF16)
        b2r = f32(inp["b2"][l])
        b2_nz.append(bool(np.any(b2r != 0)))
        if b2_nz[-1]:
            sh[f"b2row{l}"] = np.asarray(b2r[None, :], BF16)

    gf, bf_ = f32(inp["lnf_g"]), f32(inp["lnf_b"])
    wo = f32(inp["Wout"])
    sh["wout"] = np.asarray(tile3(gf[:, None] * wo, V), BF16)
    boutc = f32(inp["bout"]) + wo.T @ bf_
    bout_nz = bool(np.any(boutc != 0))
    if bout_nz:
        sh["boutc"] = boutc.reshape(V, 1)
    flags = (tuple(bq_nz), tuple(bk_nz), tuple(bv_nz), tuple(bp_nz),
             tuple(b1_nz), tuple(b2_nz), bout_nz)
    return sh, flags


def _onehot(xc):
    """xc: [BPC, T] ints -> [P, TOK] bf16 one-hot (feature-major, zero-padded
    to 128 rows so the embedding contraction uses a full partition dim)."""
    xf = np.asarray(xc, np.int64).reshape(-1)
    oh = np.zeros((P, TOK), np.float32)
    oh[xf, np.arange(TOK)] = 1.0
    return np.asarray(oh, BF16)


def _get_nc(flags):
    if flags not in _NC_CACHE:
        _NC_CACHE[flags] = _build_nc(flags)
    return _NC_CACHE[flags]


def make_in_maps(inputs):
    sh, flags = _prep_shared(inputs)
    x = np.asarray(inputs["x"])
    in_maps = []
    for c in range(NCORES):
        m = dict(sh)
        m["oh"] = _onehot(x[c * BPC:(c + 1) * BPC])
        in_maps.append(m)
    return in_maps, flags


def kernel(**inputs):
    import os
    from concourse.bass_utils import run_bass_kernel_spmd

    in_maps, flags = make_in_maps(inputs)
    nc = _get_nc(flags)
    kw = {}
    if os.environ.get("BASS_TRACE"):
        d = os.environ.get("BASS_TRACE_DIR", "/tmp/bass_trace")
        os.makedirs(d, exist_ok=True)
        kw["tmpdir"] = d
    res = run_bass_kernel_spmd(nc, in_maps, list(range(NCORES)), **kw)
    kernel._last = res
    outs = []
    for c in range(NCORES):
        lt = np.asarray(res.results[c]["logT"], np.float32)   # [V, TOK]
        outs.append(np.ascontiguousarray(lt.T).reshape(BPC, T, V))
    return np.concatenate(outs, axis=0)


kernel._last = None


# revision 40
# speedup vs baseline: 1.1827x; 1.1827x over previous
"""Trainium2 Bass kernel for a 3-layer dense transformer (BigramModel).

Contract: kernel(**inputs) takes the FULL unsharded numpy inputs (as produced
by setup_inputs) and returns the full [B, T, V] float32 logits. Internally the
batch dim B=128 is sharded 16-per-core across 8 NeuronCores (pure data
parallelism, weights replicated), one Bass/Tile NEFF run via
run_bass_kernel_spmd.

v2 design notes (vs the v1 baseline that was Sync-engine bound at 2.6ms):
  - All XBAR DMA transposes are batched 12-into-1: one dma_start_transpose per
    512-token block turns [128, 4, 384] token-major into [128, 12, 128]
    feature-major (c12 = j*3 + c). 1536 transpose instructions -> ~80.
  - Layers run as two passes (attention pass over all 8 blocks, then MLP
    pass) so LN stats + rstd are hoisted: rstd = reciprocal(sqrt(var+eps))
    costs one ACT Sqrt (table switch) + one DVE reciprocal per pass instead
    of Ln/Exp table thrash per block (real HW puts Ln and Exp in different
    ACT table sets; v1 paid 112 x 1.3us table loads).
  - proj and W2 run token-major (lhsT = transposed activations, rhs = W) so
    the residual add is one scalar_tensor_tensor from PSUM into h -- no
    reverse transposes, no separate bias/copy ops.
  - attention o accumulates both key-halves in one PSUM bank (start/stop),
    evacuated by per-head DVE reciprocal + tensor_scalar (no ACT copies).
  - softmax exp stays on ACT; W1 relu evac alternates ACT/DVE to balance.
"""

import numpy as np
import ml_dtypes

BF16 = ml_dtypes.bfloat16

P = 128
T = 256
E = 384
V = 65
H = 6
HS = 64
FF = 1536
L = 3
NCORES = 8
BPC = 16              # sequences per core
TOK = BPC * T         # 4096 tokens per core
NT = TOK // P         # 32 token tiles
NB = TOK // 512       # 8 blocks of 512 tokens (2 seqs)
ECH = E // P          # 3
FCH = FF // P         # 12

_NC_CACHE = {}
PIPE = True


def _build_nc(flags):
    """Build + compile the Bass program.

    flags = (bq_nz, bk_nz, bv_nz, bp_nz, b1_nz, b2_nz, bout_nz) with per-layer
    tuples for the first six."""
    import concourse.bacc as bacc
    import concourse.mybir as mybir
    import concourse.tile as tile

    dt = mybir.dt
    f32 = dt.float32
    bf = dt.bfloat16
    Alu = mybir.AluOpType
    Act = mybir.ActivationFunctionType

    nc = bacc.Bacc("TRN2", target_bir_lowering=False, debug=False, num_devices=1)

    bq_nz, bk_nz, bv_nz, bp_nz, b1_nz, b2_nz, bout_nz = flags

    # ---- DRAM tensors ----
    D = {}
    D["oh"] = nc.dram_tensor("oh", [P, TOK], bf, kind="ExternalInput")
    D["te"] = nc.dram_tensor("te", [V, E], bf, kind="ExternalInput")
    D["pos"] = nc.dram_tensor("pos", [P, 2, E], f32, kind="ExternalInput")
    D["mask"] = nc.dram_tensor("mask", [P, P], bf, kind="ExternalInput")
    for l in range(L):
        for w in ("wq", "wk", "wv", "wproj"):
            D[f"{w}{l}"] = nc.dram_tensor(f"{w}{l}", [P, ECH, E], bf, kind="ExternalInput")
        D[f"w1{l}"] = nc.dram_tensor(f"w1{l}", [P, ECH, FF], bf, kind="ExternalInput")
        D[f"w2{l}"] = nc.dram_tensor(f"w2{l}", [P, FCH, E], bf, kind="ExternalInput")
        if bq_nz[l]:
            D[f"bq{l}"] = nc.dram_tensor(f"bq{l}", [P, ECH], f32, kind="ExternalInput")
        if bk_nz[l]:
            D[f"bk{l}"] = nc.dram_tensor(f"bk{l}", [P, ECH], f32, kind="ExternalInput")
        if bv_nz[l]:
            D[f"bvrow{l}"] = nc.dram_tensor(f"bvrow{l}", [1, E], bf, kind="ExternalInput")
        if bp_nz[l]:
            D[f"bprow{l}"] = nc.dram_tensor(f"bprow{l}", [1, E], bf, kind="ExternalInput")
        if b1_nz[l]:
            D[f"b1c{l}"] = nc.dram_tensor(f"b1c{l}", [P, FCH], f32, kind="ExternalInput")
        if b2_nz[l]:
            D[f"b2row{l}"] = nc.dram_tensor(f"b2row{l}", [1, E], bf, kind="ExternalInput")
    D["wout"] = nc.dram_tensor("wout", [P, ECH, V], bf, kind="ExternalInput")
    if bout_nz:
        D["boutc"] = nc.dram_tensor("boutc", [V, 1], f32, kind="ExternalInput")
    D["logT"] = nc.dram_tensor("logT", [V, TOK], f32, kind="ExternalOutput")

    with tile.TileContext(nc) as tc:
        import contextlib

        with contextlib.ExitStack() as ctx:
            const = ctx.enter_context(tc.tile_pool(name="const", bufs=1))
            wpool = ctx.enter_context(tc.tile_pool(name="wpool", bufs=2))
            act = ctx.enter_context(tc.tile_pool(name="act", bufs=2))
            acts = ctx.enter_context(tc.tile_pool(name="acts", bufs=3))
            act2 = ctx.enter_context(tc.tile_pool(name="act2", bufs=2))
            act1 = ctx.enter_context(tc.tile_pool(name="act1", bufs=2))
            ps_lin = ctx.enter_context(tc.tile_pool(name="ps_lin", bufs=4, space="PSUM"))
            ps_att = ctx.enter_context(tc.tile_pool(name="ps_att", bufs=4, space="PSUM"))

            # ---- constants ----
            # the K=65 embedding contraction is padded to K=128 host-side
            # (sub-128 partition matmuls are flaky on HW); pad rows are zero.
            te_sb = const.tile([P, E], bf, tag="te")
            nc.vector.memset(te_sb[:], 0.0)
            nc.sync.dma_start(out=te_sb[0:V, :], in_=D["te"].ap())
            pos_sb = const.tile([P, 2, E], f32, tag="pos")
            nc.sync.dma_start(out=pos_sb[:], in_=D["pos"].ap())
            mask_sb = const.tile([P, P], bf, tag="mask")
            nc.sync.dma_start(out=mask_sb[:], in_=D["mask"].ap())
            boutc_sb = None
            if bout_nz:
                boutc_sb = const.tile([V, 1], f32, tag="boutc")
                nc.sync.dma_start(out=boutc_sb[:], in_=D["boutc"].ap())
            ones_sb = const.tile([1, P], bf, tag="ones")
            nc.vector.memset(ones_sb[:], 1.0)
            eps_sb = const.tile([P, 1], f32, tag="eps")
            nc.vector.memset(eps_sb[:], 1e-5)
            zero_sb = const.tile([P, 1], f32, tag="zero")
            nc.vector.memset(zero_sb[:], 0.0)

            # persistent residual tiles (token-major fp32)
            h = [const.tile([P, E], f32, tag=f"h{i}", name=f"h{i}") for i in range(NT)]

            # ---- LN stats machinery (pipelined: sum(h) rides the accum_out
            # of each tile's LAST residual-update STT inside the previous
            # pass; only sum(h^2) costs an extra DVE op per tile; the cheap
            # finish step runs at pass start) ----

            def new_stats(tag):
                """Allocate (S, SS) accumulators for one LN pass."""
                s = const.tile([P, NT, 2], f32, tag=f"sv_{tag}", name=f"sv_{tag}")
                return s

            def emit_stats(sv, i0, n=4):
                """sum(h^2) for tiles i0..i0+n-1 (sum(h) already accumulated)."""
                for i in range(i0, i0 + n):
                    dm = acts.tile([P, E], f32, tag="stat_dm")
                    nc.vector.scalar_tensor_tensor(
                        out=dm[:], in0=h[i][:], scalar=0.0, in1=h[i][:],
                        op0=Alu.add, op1=Alu.mult, accum_out=sv[:, i, 1:2],
                    )

            def finish_stats(sv, tag):
                """(S, SS) -> mean [P,NT], rstd [P,NT]."""
                m = const.tile([P, NT], f32, tag=f"m_{tag}", name=f"m_{tag}")
                nc.vector.tensor_scalar_mul(out=m[:], in0=sv[:, :, 0], scalar1=1.0 / E)
                var = const.tile([P, NT], f32, tag=f"va_{tag}", name=f"va_{tag}")
                nc.vector.scalar_tensor_tensor(
                    out=var[:], in0=m[:], scalar=0.0, in1=m[:],
                    op0=Alu.add, op1=Alu.mult,
                )
                nc.vector.scalar_tensor_tensor(
                    out=var[:], in0=sv[:, :, 1], scalar=1.0 / E, in1=var[:],
                    op0=Alu.mult, op1=Alu.subtract,
                )
                sd = acts.tile([P, NT], f32, tag="sd")
                nc.scalar.activation(
                    out=sd[:], in_=var[:], func=Act.Sqrt, bias=eps_sb[:],
                )
                rstd = const.tile([P, NT], f32, tag=f"rs_{tag}", name=f"rs_{tag}")
                nc.vector.reciprocal(out=rstd[:], in_=sd[:])
                return m, rstd

            def load_w(name, shape, dtp, tag=None):
                t = wpool.tile(shape, dtp, tag=tag or name[:-1])
                nc.sync.dma_start(out=t[:], in_=D[name].ap())
                return t

            # layer-0 weights go on the DMA queue before the embedding so the
            # first attention pass never stalls on them (the v4 trace showed
            # ~12us of PE idle waiting for wq/wk at t~38us).
            _W0 = {}
            for _w, _shp in (("wq", [P, ECH, E]), ("wk", [P, ECH, E]),
                             ("wv", [P, ECH, E]), ("wproj", [P, ECH, E]),
                             ("w1", [P, ECH, FF]), ("w2", [P, FCH, E])):
                _W0[_w] = load_w(f"{_w}0", _shp, bf)

            # ---- embedding: h = onehot.T @ tok_emb + pos ----
            sv1 = new_stats("l0a")
            for blk in range(NB):
                ohc = act2.tile([P, 512], bf, tag="ohc")
                nc.sync.dma_start(
                    out=ohc[:], in_=D["oh"].ap()[:, blk * 512:(blk + 1) * 512])
                for jj in range(4):
                    i = 4 * blk + jj
                    ps = ps_lin.tile([P, 512], f32, tag="mm")
                    nc.tensor.matmul(
                        ps[:, 0:E], ohc[:, jj * P:(jj + 1) * P], te_sb[:],
                        start=True, stop=True,
                    )
                    nc.vector.scalar_tensor_tensor(
                        out=h[i][:], in0=ps[:, 0:E], scalar=0.0,
                        in1=pos_sb[:, i % 2, :], op0=Alu.add, op1=Alu.add,
                        accum_out=sv1[:, i, 0:1],
                    )
                emit_stats(sv1, 4 * blk)

            def make_xnT(i0, m, rstd):
                """xn = (h - m) * rstd for 4 tiles -> single batched transpose
                to feature-major [P, 12, 128] (c12 = j*3 + c)."""
                xn4 = act.tile([P, 4, E], bf, tag="xn4")
                for j in range(4):
                    nc.vector.tensor_scalar(
                        out=xn4[:, j, :], in0=h[i0 + j][:],
                        scalar1=m[:, i0 + j:i0 + j + 1],
                        scalar2=rstd[:, i0 + j:i0 + j + 1],
                        op0=Alu.subtract, op1=Alu.mult,
                    )
                xnT = act.tile([P, 12, P], bf, tag="xnT")
                nc.sync.dma_start_transpose(
                    xnT[:], xn4[:].rearrange("p a b -> p (a b)"))
                return xnT

            def lin_fmaj(xnT, w_sb, bias_col, fch, tag, evac, pool=None):
                """feature-major out [P, fch, 512] bf16; evac in {dve, act, mixN}."""
                o = (pool or act).tile([P, fch, 512], bf, tag=tag, name=tag)
                rhs_view = xnT[:].rearrange("p (j c) a -> p c j a", c=ECH)
                for f in range(fch):
                    ps = ps_lin.tile([P, 512], f32, tag="mm")
                    for c in range(ECH):
                        nc.tensor.matmul(
                            ps[:], w_sb[:, c, f * P:(f + 1) * P], rhs_view[:, c],
                            start=(c == 0), stop=(c == ECH - 1),
                        )
                    use_act = (evac == "act") or (evac == "mix" and f % 2 == 0)
                    if use_act:
                        if bias_col is not None:
                            nc.scalar.activation(
                                out=o[:, f, :], in_=ps[:], func=Act.Copy,
                                bias=bias_col[:, f:f + 1])
                        else:
                            nc.scalar.copy(out=o[:, f, :], in_=ps[:])
                    else:
                        if bias_col is not None:
                            nc.vector.tensor_scalar_add(
                                out=o[:, f, :], in0=ps[:],
                                scalar1=bias_col[:, f:f + 1])
                        else:
                            nc.vector.tensor_copy(out=o[:, f, :], in_=ps[:])
                return o

            def lin_fmaj_relu(xnT, w_sb, bias_col, tag):
                """W1 + relu, evac alternating ACT/DVE."""
                o = act1.tile([P, FCH, 512], bf, tag=tag, name=tag)
                rhs_view = xnT[:].rearrange("p (j c) a -> p c j a", c=ECH)
                for f in range(FCH):
                    ps = ps_lin.tile([P, 512], f32, tag="mm")
                    for c in range(ECH):
                        nc.tensor.matmul(
                            ps[:], w_sb[:, c, f * P:(f + 1) * P], rhs_view[:, c],
                            start=(c == 0), stop=(c == ECH - 1),
                        )
                    if f % 2 == 0:
                        nc.scalar.activation(
                            out=o[:, f, :], in_=ps[:], func=Act.Relu,
                            bias=(bias_col[:, f:f + 1] if bias_col is not None else 0.0))
                    else:
                        if bias_col is not None:
                            nc.vector.tensor_scalar(
                                out=o[:, f, :], in0=ps[:],
                                scalar1=bias_col[:, f:f + 1], scalar2=zero_sb[:],
                                op0=Alu.add, op1=Alu.max,
                            )
                        else:
                            nc.vector.tensor_scalar_max(
                                out=o[:, f, :], in0=ps[:], scalar1=zero_sb[:],
                            )
                return o

            def lin_tmaj_resid(xT, w_sb, nch, brow, i0, sv=None):
                """h[i0+j] += xT_j @ W + brow, token-major: one STT per tile.
                If sv is given, the STT also accumulates sum(h_new) into
                sv[:, i, 0:1] (the next LN pass's S statistic)."""
                for j in range(4):
                    ps = ps_lin.tile([P, 512], f32, tag="mm")
                    for c in range(nch):
                        nc.tensor.matmul(
                            ps[:, 0:E], xT[:, j * nch + c, :] if nch == ECH
                            else xT[:, c, j * P:(j + 1) * P],
                            w_sb[:, c, :],
                            start=(c == 0),
                            stop=(c == nch - 1 and brow is None),
                        )
                    if brow is not None:
                        nc.tensor.matmul(
                            ps[:, 0:E], ones_sb[:], brow[:], start=False, stop=True,
                        )
                    nc.vector.scalar_tensor_tensor(
                        out=h[i0 + j][:], in0=ps[:, 0:E], scalar=0.0,
                        in1=h[i0 + j][:], op0=Alu.add, op1=Alu.add,
                        accum_out=(None if sv is None else sv[:, i0 + j, 0:1]),
                    )

            def load_w(name, shape, dtp, tag=None):
                t = wpool.tile(shape, dtp, tag=tag or name[:-1])
                nc.sync.dma_start(out=t[:], in_=D[name].ap())
                return t

            scale = float(HS) ** -0.5

            # ---- transformer layers ----
            for l in range(L):
                if l == 0:
                    wq, wk, wv = _W0["wq"], _W0["wk"], _W0["wv"]
                    wproj, w1, w2 = _W0["wproj"], _W0["w1"], _W0["w2"]
                else:
                    wq = load_w(f"wq{l}", [P, ECH, E], bf)
                    wk = load_w(f"wk{l}", [P, ECH, E], bf)
                    wv = load_w(f"wv{l}", [P, ECH, E], bf)
                    wproj = load_w(f"wproj{l}", [P, ECH, E], bf)
                    w1 = load_w(f"w1{l}", [P, ECH, FF], bf)
                    w2 = load_w(f"w2{l}", [P, FCH, E], bf)
                bq = load_w(f"bq{l}", [P, ECH], f32) if bq_nz[l] else None
                bk = load_w(f"bk{l}", [P, ECH], f32) if bk_nz[l] else None
                bvrow = load_w(f"bvrow{l}", [1, E], bf) if bv_nz[l] else None
                bprow = load_w(f"bprow{l}", [1, E], bf) if bp_nz[l] else None
                b1c = load_w(f"b1c{l}", [P, FCH], f32) if b1_nz[l] else None
                b2row = load_w(f"b2row{l}", [1, E], bf) if b2_nz[l] else None

                # ======== attention pass ========
                m1, rstd1 = finish_stats(sv1, f"a{l}")
                sv2 = new_stats(f"m{l}")
                pend = None   # deferred proj of the previous block
                for b in range(NB):
                    i0 = 4 * b
                    xnT = make_xnT(i0, m1, rstd1)
                    QT = lin_fmaj(xnT, wq, bq, ECH, "QT", "dve")
                    KT = lin_fmaj(xnT, wk, bk, ECH, "KT", "act")
                    # V token-major, ones-augmented: [P, 4, H, 65]
                    Vt = act.tile([P, 4, H, 65], bf, tag="Vt")
                    for j in range(4):
                        ps = ps_lin.tile([P, 512], f32, tag="mm")
                        for c in range(ECH):
                            nc.tensor.matmul(
                                ps[:, 0:E], xnT[:, j * ECH + c, :], wv[:, c, :],
                                start=(c == 0),
                                stop=(c == ECH - 1 and bvrow is None),
                            )
                        if bvrow is not None:
                            nc.tensor.matmul(
                                ps[:, 0:E], ones_sb[:], bvrow[:],
                                start=False, stop=True,
                            )
                        nc.vector.tensor_copy(
                            out=Vt[:, j, :, 0:64],
                            in_=ps[:, 0:E].rearrange("p (h d) -> p h d", h=H),
                        )
                        nc.vector.memset(Vt[:, j, :, 64:65], 1.0)

                    # scores + exp + mask for BOTH sequences first (dense PE
                    # stream; exp latency hidden by the deferred proj below).
                    # scores are packed 2 heads (st=0) / 4 heads (st=1) per
                    # PSUM bank so each exp is one wide ACT instruction.
                    probs2 = []
                    for s in range(2):
                        tb = s * 256
                        probs = act2.tile([P, 2, H, 256], bf, tag="probs")
                        def score_mm(sc_reg, hh, st):
                            tlo = 128 if st == 1 else 0
                            c, off = divmod(hh * HS, P)
                            nc.tensor.matmul(
                                sc_reg,
                                KT[off:off + HS, c, tb + st * P: tb + (st + 1) * P],
                                QT[off:off + HS, c, tb + tlo: tb + 256],
                                start=True, stop=True,
                            )
                        for st in range(2):
                            tlo = 128 if st == 1 else 0
                            for hh in range(H):
                                sc = ps_att.tile([P, 512], f32, tag="att", name="sc")
                                score_mm(sc[:, 0:256 - tlo], hh, st)
                                nc.scalar.activation(
                                    out=probs[:, st, hh, tlo:256],
                                    in_=sc[:, 0:256 - tlo],
                                    func=Act.Exp, scale=scale,
                                )
                        # causal mask: only the diagonal 128x128 needs it
                        for st in range(2):
                            tlo = 128 if st == 1 else 0
                            nc.vector.tensor_tensor(
                                out=probs[:, st, :, tlo:tlo + P],
                                in0=probs[:, st, :, tlo:tlo + P],
                                in1=mask_sb[:, None, :].to_broadcast((P, H, P)),
                                op=Alu.mult,
                            )
                        probs2.append(probs)

                    # deferred proj of block b-1 fills the PE while exp runs
                    if PIPE and pend is not None:
                        lin_tmaj_resid(pend[1], wproj, ECH, bprow, pend[0], sv2)
                        emit_stats(sv2, pend[0])

                    # o matmuls packed 4+2 heads per PSUM bank (all K=128,
                    # strict-FIFO drains -> no same-bank write races); softmax
                    # divide is one reciprocal + one broadcast TT per bank.
                    onorm4 = act.tile([P, 4, E], bf, tag="onorm4")
                    for s in range(2):
                        probs = probs2[s]
                        for tt in range(2):  # query tiles of this seq
                            for hg, nh in ((0, 4), (4, 2)):
                                po = ps_att.tile([P, 512], f32, tag="att", name="po")
                                for k in range(nh):
                                    hh = hg + k
                                    dst = po[:, k * P:k * P + 65]
                                    if tt == 0:
                                        nc.tensor.matmul(
                                            dst, probs[:, 0, hh, 0:P],
                                            Vt[:, 2 * s, hh, :],
                                            start=True, stop=True,
                                        )
                                    else:
                                        nc.tensor.matmul(
                                            dst, probs[:, 0, hh, P:256],
                                            Vt[:, 2 * s, hh, :],
                                            start=True, stop=False,
                                        )
                                        nc.tensor.matmul(
                                            dst, probs[:, 1, hh, P:256],
                                            Vt[:, 2 * s + 1, hh, :],
                                            start=False, stop=True,
                                        )
                                pv = po[:].rearrange("p (k c) -> p k c", c=P)
                                rec = acts.tile([P, 4], f32, tag="rec")
                                nc.vector.reciprocal(
                                    out=rec[:, 0:nh], in_=pv[:, 0:nh, 64:65])
                                nc.vector.tensor_tensor(
                                    out=onorm4[:, 2 * s + tt, hg * 64:(hg + nh) * 64]
                                    .rearrange("p (k d) -> p k d", d=64),
                                    in0=pv[:, 0:nh, 0:64],
                                    in1=rec[:, 0:nh, None].to_broadcast((P, nh, 64)),
                                    op=Alu.mult,
                                )
                    oT = act.tile([P, 12, P], bf, tag="oT")
                    nc.sync.dma_start_transpose(
                        oT[:], onorm4[:].rearrange("p a b -> p (a b)"))
                    if PIPE:
                        pend = (i0, oT)
                    else:
                        lin_tmaj_resid(oT, wproj, ECH, bprow, i0, sv2)
                        emit_stats(sv2, i0)
                if PIPE:
                    lin_tmaj_resid(pend[1], wproj, ECH, bprow, pend[0], sv2)
                    emit_stats(sv2, pend[0])

                # ======== MLP pass ========
                m2, rstd2 = finish_stats(sv2, f"m{l}")
                sv1 = new_stats("f" if l == L - 1 else f"a{l + 1}")
                pend = None   # deferred W2 of the previous block
                for b in range(NB):
                    i0 = 4 * b
                    xnT = make_xnT(i0, m2, rstd2)
                    aT = lin_fmaj_relu(xnT, w1, b1c, "aT")
                    if PIPE and pend is not None:
                        lin_tmaj_resid(pend[1], w2, FCH, b2row, pend[0], sv1)
                        emit_stats(sv1, pend[0])
                    if PIPE:
                        pend = (i0, aT)
                    else:
                        lin_tmaj_resid(aT, w2, FCH, b2row, i0, sv1)
                        emit_stats(sv1, i0)
                if PIPE:
                    lin_tmaj_resid(pend[1], w2, FCH, b2row, pend[0], sv1)
                    emit_stats(sv1, pend[0])

            # ---- final LN + unembed (feature-major logits) ----
            wout = wpool.tile([P, ECH, V], bf, tag="wout")
            nc.sync.dma_start(out=wout[:], in_=D["wout"].ap())
            mf, rstdf = finish_stats(sv1, "f")
            for b in range(NB):
                xnT = make_xnT(4 * b, mf, rstdf)
                rhs_view = xnT[:].rearrange("p (j c) a -> p c j a", c=ECH)
                ps = ps_lin.tile([V, 512], f32, tag="mm", name="mmv")
                for c in range(ECH):
                    nc.tensor.matmul(
                        ps[:], wout[:, c, :], rhs_view[:, c],
                        start=(c == 0), stop=(c == ECH - 1),
                    )
                lt = act2.tile([V, 512], f32, tag="lt")
                if boutc_sb is not None:
                    nc.vector.tensor_scalar_add(
                        out=lt[:], in0=ps[:], scalar1=boutc_sb[:])
                else:
                    nc.vector.tensor_copy(out=lt[:], in_=ps[:])
                nc.sync.dma_start(
                    out=D["logT"].ap()[:, b * 512:(b + 1) * 512], in_=lt[:],
                )

    nc.compile()
    return nc


def _prep_shared(inp):
    """Host-side weight prep: layout rearrangement + LN gamma/beta folding."""
    sh = {}

    def f32(x):
        return np.asarray(x, np.float32)

    sh["te"] = np.asarray(f32(inp["tok_emb"]), BF16)                      # [V,E]
    sh["pos"] = np.ascontiguousarray(
        f32(inp["pos_emb"]).reshape(2, P, E).transpose(1, 0, 2))          # [P,2,E]
    sh["mask"] = np.asarray(np.triu(np.ones((P, P), np.float32)), BF16)   # [P,P]

    def tile3(w, fdim):  # [E, fdim] -> [P, ECH, fdim]
        return np.ascontiguousarray(w.reshape(ECH, P, fdim).transpose(1, 0, 2))

    def col(b, nch):  # [nch*P] -> [P, nch]
        return np.ascontiguousarray(b.reshape(nch, P).T)

    bq_nz, bk_nz, bv_nz, bp_nz, b1_nz, b2_nz = [], [], [], [], [], []
    for l in range(L):
        g1, b1_ = f32(inp["ln1_g"][l]), f32(inp["ln1_b"][l])
        g2, b2_ = f32(inp["ln2_g"][l]), f32(inp["ln2_b"][l])
        wq = f32(inp["Wq"][l]).transpose(1, 0, 2).reshape(E, E)   # head-major cols
        wk = f32(inp["Wk"][l]).transpose(1, 0, 2).reshape(E, E)
        wv = f32(inp["Wv"][l]).transpose(1, 0, 2).reshape(E, E)
        sh[f"wq{l}"] = np.asarray(tile3(g1[:, None] * wq, E), BF16)
        sh[f"wk{l}"] = np.asarray(tile3(g1[:, None] * wk, E), BF16)
        sh[f"wv{l}"] = np.asarray(tile3(g1[:, None] * wv, E), BF16)
        bq = wq.T @ b1_
        bk = wk.T @ b1_
        bv = wv.T @ b1_
        bq_nz.append(bool(np.any(bq != 0)))
        bk_nz.append(bool(np.any(bk != 0)))
        bv_nz.append(bool(np.any(bv != 0)))
        if bq_nz[-1]:
            sh[f"bq{l}"] = col(bq, ECH)
        if bk_nz[-1]:
            sh[f"bk{l}"] = col(bk, ECH)
        if bv_nz[-1]:
            sh[f"bvrow{l}"] = np.asarray(bv[None, :], BF16)
        wp = f32(inp["Wproj"][l])
        sh[f"wproj{l}"] = np.asarray(tile3(wp, E), BF16)
        bp = f32(inp["bproj"][l])
        bp_nz.append(bool(np.any(bp != 0)))
        if bp_nz[-1]:
            sh[f"bprow{l}"] = np.asarray(bp[None, :], BF16)
        w1 = f32(inp["W1"][l])
        sh[f"w1{l}"] = np.asarray(tile3(g2[:, None] * w1, FF), BF16)
        b1ff = f32(inp["b1"][l]) + w1.T @ b2_
        b1_nz.append(bool(np.any(b1ff != 0)))
        if b1_nz[-1]:
            sh[f"b1c{l}"] = col(b1ff, FCH)
        w2 = f32(inp["W2"][l])
        sh[f"w2{l}"] = np.asarray(
            w2.reshape(FCH, P, E).transpose(1, 0, 2), BF16)
        b2r = f32(inp["b2"][l])
        b2_nz.append(bool(np.any(b2r != 0)))
        if b2_nz[-1]:
            sh[f"b2row{l}"] = np.asarray(b2r[None, :], BF16)

    gf, bf_ = f32(inp["lnf_g"]), f32(inp["lnf_b"])
    wo = f32(inp["Wout"])
    sh["wout"] = np.asarray(tile3(gf[:, None] * wo, V), BF16)
    boutc = f32(inp["bout"]) + wo.T @ bf_
    bout_nz = bool(np.any(boutc != 0))
    if bout_nz:
        sh["boutc"] = boutc.reshape(V, 1)
    flags = (tuple(bq_nz), tuple(bk_nz), tuple(bv_nz), tuple(bp_nz),
             tuple(b1_nz), tuple(b2_nz), bout_nz)
    return sh, flags


def _onehot(xc):
    """xc: [BPC, T] ints -> [P, TOK] bf16 one-hot (feature-major, zero-padded
    to 128 rows so the embedding contraction uses a full partition dim)."""
    xf = np.asarray(xc, np.int64).reshape(-1)
    oh = np.zeros((P, TOK), np.float32)
    oh[xf, np.arange(TOK)] = 1.0
    return np.asarray(oh, BF16)


def _get_nc(flags):
    if flags not in _NC_CACHE:
        _NC_CACHE[flags] = _build_nc(flags)
    return _NC_CACHE[flags]


def make_in_maps(inputs):
    sh, flags = _prep_shared(inputs)
    x = np.asarray(inputs["x"])
    in_maps = []
    for c in range(NCORES):
        m = dict(sh)
        m["oh"] = _onehot(x[c * BPC:(c + 1) * BPC])
        in_maps.append(m)
    return in_maps, flags


def kernel(**inputs):
    import os
    from concourse.bass_utils import run_bass_kernel_spmd

    in_maps, flags = make_in_maps(inputs)
    nc = _get_nc(flags)
    kw = {}
    if os.environ.get("BASS_TRACE"):
        d = os.environ.get("BASS_TRACE_DIR", "/tmp/bass_trace")
        os.makedirs(d, exist_ok=True)
        kw["tmpdir"] = d
    res = run_bass_kernel_spmd(nc, in_maps, list(range(NCORES)), **kw)
    kernel._last = res
    outs = []
    for c in range(NCORES):
        lt = np.asarray(res.results[c]["logT"], np.float32)   # [V, TOK]
        outs.append(np.ascontiguousarray(lt.T).reshape(BPC, T, V))
    return np.concatenate(outs, axis=0)


kernel._last = None


# revision 44
# speedup vs baseline: 1.2382x; 1.0469x over previous
"""Trainium2 Bass kernel for a 3-layer dense transformer (BigramModel).

Contract: kernel(**inputs) takes the FULL unsharded numpy inputs (as produced
by setup_inputs) and returns the full [B, T, V] float32 logits. Internally the
batch dim B=128 is sharded 16-per-core across 8 NeuronCores (pure data
parallelism, weights replicated), one Bass/Tile NEFF run via
run_bass_kernel_spmd.

v2 design notes (vs the v1 baseline that was Sync-engine bound at 2.6ms):
  - All XBAR DMA transposes are batched 12-into-1: one dma_start_transpose per
    512-token block turns [128, 4, 384] token-major into [128, 12, 128]
    feature-major (c12 = j*3 + c). 1536 transpose instructions -> ~80.
  - Layers run as two passes (attention pass over all 8 blocks, then MLP
    pass) so LN stats + rstd are hoisted: rstd = reciprocal(sqrt(var+eps))
    costs one ACT Sqrt (table switch) + one DVE reciprocal per pass instead
    of Ln/Exp table thrash per block (real HW puts Ln and Exp in different
    ACT table sets; v1 paid 112 x 1.3us table loads).
  - proj and W2 run token-major (lhsT = transposed activations, rhs = W) so
    the residual add is one scalar_tensor_tensor from PSUM into h -- no
    reverse transposes, no separate bias/copy ops.
  - attention o accumulates both key-halves in one PSUM bank (start/stop),
    evacuated by per-head DVE reciprocal + tensor_scalar (no ACT copies).
  - softmax exp stays on ACT; W1 relu evac alternates ACT/DVE to balance.
"""

import numpy as np
import ml_dtypes

BF16 = ml_dtypes.bfloat16

P = 128
T = 256
E = 384
V = 65
H = 6
HS = 64
FF = 1536
L = 3
NCORES = 8
BPC = 16              # sequences per core
TOK = BPC * T         # 4096 tokens per core
NT = TOK // P         # 32 token tiles
NB = TOK // 512       # 8 blocks of 512 tokens (2 seqs)
ECH = E // P          # 3
FCH = FF // P         # 12

_NC_CACHE = {}
PIPE = True


def _build_nc(flags):
    """Build + compile the Bass program.

    flags = (bq_nz, bk_nz, bv_nz, bp_nz, b1_nz, b2_nz, bout_nz) with per-layer
    tuples for the first six."""
    import concourse.bacc as bacc
    import concourse.mybir as mybir
    import concourse.tile as tile

    dt = mybir.dt
    f32 = dt.float32
    bf = dt.bfloat16
    Alu = mybir.AluOpType
    Act = mybir.ActivationFunctionType

    nc = bacc.Bacc("TRN2", target_bir_lowering=False, debug=False, num_devices=1)

    bq_nz, bk_nz, bv_nz, bp_nz, b1_nz, b2_nz, bout_nz = flags

    # ---- DRAM tensors ----
    D = {}
    D["oh"] = nc.dram_tensor("oh", [P, TOK], bf, kind="ExternalInput")
    D["te"] = nc.dram_tensor("te", [V, E], bf, kind="ExternalInput")
    D["pos"] = nc.dram_tensor("pos", [P, 2, E], f32, kind="ExternalInput")
    D["mask"] = nc.dram_tensor("mask", [P, P], bf, kind="ExternalInput")
    for l in range(L):
        for w in ("wq", "wk", "wv", "wproj"):
            D[f"{w}{l}"] = nc.dram_tensor(f"{w}{l}", [P, ECH, E], bf, kind="ExternalInput")
        D[f"w1{l}"] = nc.dram_tensor(f"w1{l}", [P, ECH, FF], bf, kind="ExternalInput")
        D[f"w2{l}"] = nc.dram_tensor(f"w2{l}", [P, FCH, E], bf, kind="ExternalInput")
        if bq_nz[l]:
            D[f"bq{l}"] = nc.dram_tensor(f"bq{l}", [P, ECH], f32, kind="ExternalInput")
        if bk_nz[l]:
            D[f"bk{l}"] = nc.dram_tensor(f"bk{l}", [P, ECH], f32, kind="ExternalInput")
        if bv_nz[l]:
            D[f"bvrow{l}"] = nc.dram_tensor(f"bvrow{l}", [1, E], bf, kind="ExternalInput")
        if bp_nz[l]:
            D[f"bprow{l}"] = nc.dram_tensor(f"bprow{l}", [1, E], bf, kind="ExternalInput")
        if b1_nz[l]:
            D[f"b1c{l}"] = nc.dram_tensor(f"b1c{l}", [P, FCH], f32, kind="ExternalInput")
        if b2_nz[l]:
            D[f"b2row{l}"] = nc.dram_tensor(f"b2row{l}", [1, E], bf, kind="ExternalInput")
    D["wout"] = nc.dram_tensor("wout", [P, ECH, V], bf, kind="ExternalInput")
    if bout_nz:
        D["boutc"] = nc.dram_tensor("boutc", [V, 1], f32, kind="ExternalInput")
    D["logT"] = nc.dram_tensor("logT", [V, TOK], f32, kind="ExternalOutput")

    with tile.TileContext(nc) as tc:
        import contextlib

        with contextlib.ExitStack() as ctx:
            const = ctx.enter_context(tc.tile_pool(name="const", bufs=1))
            wpool = ctx.enter_context(tc.tile_pool(name="wpool", bufs=2))
            act = ctx.enter_context(tc.tile_pool(name="act", bufs=2))
            acts = ctx.enter_context(tc.tile_pool(name="acts", bufs=3))
            act2 = ctx.enter_context(tc.tile_pool(name="act2", bufs=2))
            act1 = ctx.enter_context(tc.tile_pool(name="act1", bufs=2))
            ps_lin = ctx.enter_context(tc.tile_pool(name="ps_lin", bufs=4, space="PSUM"))
            ps_att = ctx.enter_context(tc.tile_pool(name="ps_att", bufs=4, space="PSUM"))

            # ---- constants ----
            # the K=65 embedding contraction is padded to K=128 host-side
            # (sub-128 partition matmuls are flaky on HW); pad rows are zero.
            te_sb = const.tile([P, E], bf, tag="te")
            nc.vector.memset(te_sb[:], 0.0)
            nc.sync.dma_start(out=te_sb[0:V, :], in_=D["te"].ap())
            pos_sb = const.tile([P, 2, E], f32, tag="pos")
            nc.sync.dma_start(out=pos_sb[:], in_=D["pos"].ap())
            mask_sb = const.tile([P, P], bf, tag="mask")
            nc.sync.dma_start(out=mask_sb[:], in_=D["mask"].ap())
            boutc_sb = None
            if bout_nz:
                boutc_sb = const.tile([V, 1], f32, tag="boutc")
                nc.sync.dma_start(out=boutc_sb[:], in_=D["boutc"].ap())
            ones_sb = const.tile([1, P], bf, tag="ones")
            nc.vector.memset(ones_sb[:], 1.0)
            eps_sb = const.tile([P, 1], f32, tag="eps")
            nc.vector.memset(eps_sb[:], 1e-5)
            zero_sb = const.tile([P, 1], f32, tag="zero")
            nc.vector.memset(zero_sb[:], 0.0)

            # persistent residual tiles (token-major fp32)
            h = [const.tile([P, E], f32, tag=f"h{i}", name=f"h{i}") for i in range(NT)]

            # ---- LN stats machinery (pipelined: sum(h) rides the accum_out
            # of each tile's LAST residual-update STT inside the previous
            # pass; only sum(h^2) costs an extra DVE op per tile; the cheap
            # finish step runs at pass start) ----

            def new_stats(tag):
                """Allocate (S, SS) accumulators for one LN pass."""
                s = const.tile([P, NT, 2], f32, tag=f"sv_{tag}", name=f"sv_{tag}")
                return s

            def emit_stats(sv, i0, n=4):
                """sum(h^2) for tiles i0..i0+n-1 (sum(h) already accumulated)."""
                for i in range(i0, i0 + n):
                    dm = acts.tile([P, E], f32, tag="stat_dm")
                    nc.vector.scalar_tensor_tensor(
                        out=dm[:], in0=h[i][:], scalar=0.0, in1=h[i][:],
                        op0=Alu.add, op1=Alu.mult, accum_out=sv[:, i, 1:2],
                    )

            def finish_stats(sv, tag):
                """(S, SS) -> mean [P,NT], rstd [P,NT]."""
                m = const.tile([P, NT], f32, tag=f"m_{tag}", name=f"m_{tag}")
                nc.vector.tensor_scalar_mul(out=m[:], in0=sv[:, :, 0], scalar1=1.0 / E)
                var = const.tile([P, NT], f32, tag=f"va_{tag}", name=f"va_{tag}")
                nc.vector.scalar_tensor_tensor(
                    out=var[:], in0=m[:], scalar=0.0, in1=m[:],
                    op0=Alu.add, op1=Alu.mult,
                )
                nc.vector.scalar_tensor_tensor(
                    out=var[:], in0=sv[:, :, 1], scalar=1.0 / E, in1=var[:],
                    op0=Alu.mult, op1=Alu.subtract,
                )
                sd = acts.tile([P, NT], f32, tag="sd")
                nc.scalar.activation(
                    out=sd[:], in_=var[:], func=Act.Sqrt, bias=eps_sb[:],
                )
                rstd = const.tile([P, NT], f32, tag=f"rs_{tag}", name=f"rs_{tag}")
                nc.vector.reciprocal(out=rstd[:], in_=sd[:])
                return m, rstd

            # ---- embedding: h = onehot.T @ tok_emb + pos ----
            sv1 = new_stats("l0a")
            for blk in range(NB):
                ohc = act2.tile([P, 512], bf, tag="ohc")
                nc.sync.dma_start(
                    out=ohc[:], in_=D["oh"].ap()[:, blk * 512:(blk + 1) * 512])
                for jj in range(4):
                    i = 4 * blk + jj
                    ps = ps_lin.tile([P, 512], f32, tag="mm")
                    nc.tensor.matmul(
                        ps[:, 0:E], ohc[:, jj * P:(jj + 1) * P], te_sb[:],
                        start=True, stop=True,
                    )
                    nc.vector.scalar_tensor_tensor(
                        out=h[i][:], in0=ps[:, 0:E], scalar=0.0,
                        in1=pos_sb[:, i % 2, :], op0=Alu.add, op1=Alu.add,
                        accum_out=sv1[:, i, 0:1],
                    )
                emit_stats(sv1, 4 * blk)

            def make_xnT(i0, m, rstd):
                """xn = (h - m) * rstd for 4 tiles -> single batched transpose
                to feature-major [P, 12, 128] (c12 = j*3 + c)."""
                xn4 = act.tile([P, 4, E], bf, tag="xn4")
                for j in range(4):
                    nc.vector.tensor_scalar(
                        out=xn4[:, j, :], in0=h[i0 + j][:],
                        scalar1=m[:, i0 + j:i0 + j + 1],
                        scalar2=rstd[:, i0 + j:i0 + j + 1],
                        op0=Alu.subtract, op1=Alu.mult,
                    )
                xnT = act.tile([P, 12, P], bf, tag="xnT")
                nc.sync.dma_start_transpose(
                    xnT[:], xn4[:].rearrange("p a b -> p (a b)"))
                return xnT

            def lin_fmaj(xnT, w_sb, bias_col, fch, tag, evac, pool=None):
                """feature-major out [P, fch, 512] bf16; evac in {dve, act, mixN}."""
                o = (pool or act).tile([P, fch, 512], bf, tag=tag, name=tag)
                rhs_view = xnT[:].rearrange("p (j c) a -> p c j a", c=ECH)
                for f in range(fch):
                    ps = ps_lin.tile([P, 512], f32, tag="mm")
                    for c in range(ECH):
                        nc.tensor.matmul(
                            ps[:], w_sb[:, c, f * P:(f + 1) * P], rhs_view[:, c],
                            start=(c == 0), stop=(c == ECH - 1),
                        )
                    use_act = (evac == "act") or (evac == "mix" and f % 2 == 0)
                    if use_act:
                        if bias_col is not None:
                            nc.scalar.activation(
                                out=o[:, f, :], in_=ps[:], func=Act.Copy,
                                bias=bias_col[:, f:f + 1])
                        else:
                            nc.scalar.copy(out=o[:, f, :], in_=ps[:])
                    else:
                        if bias_col is not None:
                            nc.vector.tensor_scalar_add(
                                out=o[:, f, :], in0=ps[:],
                                scalar1=bias_col[:, f:f + 1])
                        else:
                            nc.vector.tensor_copy(out=o[:, f, :], in_=ps[:])
                return o

            def lin_fmaj_relu(xnT, w_sb, bias_col, tag):
                """W1 + relu, evac alternating ACT/DVE."""
                o = act1.tile([P, FCH, 512], bf, tag=tag, name=tag)
                rhs_view = xnT[:].rearrange("p (j c) a -> p c j a", c=ECH)
                for f in range(FCH):
                    ps = ps_lin.tile([P, 512], f32, tag="mm")
                    for c in range(ECH):
                        nc.tensor.matmul(
                            ps[:], w_sb[:, c, f * P:(f + 1) * P], rhs_view[:, c],
                            start=(c == 0), stop=(c == ECH - 1),
                        )
                    if f % 2 == 0:
                        nc.scalar.activation(
                            out=o[:, f, :], in_=ps[:], func=Act.Relu,
                            bias=(bias_col[:, f:f + 1] if bias_col is not None else 0.0))
                    else:
                        if bias_col is not None:
                            nc.vector.tensor_scalar(
                                out=o[:, f, :], in0=ps[:],
                                scalar1=bias_col[:, f:f + 1], scalar2=zero_sb[:],
                                op0=Alu.add, op1=Alu.max,
                            )
                        else:
                            nc.vector.tensor_scalar_max(
                                out=o[:, f, :], in0=ps[:], scalar1=zero_sb[:],
                            )
                return o

            def lin_tmaj_resid(xT, w_sb, nch, brow, i0, sv=None):
                """h[i0+j] += xT_j @ W + brow, token-major: one STT per tile.
                If sv is given, the STT also accumulates sum(h_new) into
                sv[:, i, 0:1] (the next LN pass's S statistic)."""
                for j in range(4):
                    ps = ps_lin.tile([P, 512], f32, tag="mm")
                    for c in range(nch):
                        nc.tensor.matmul(
                            ps[:, 0:E], xT[:, j * nch + c, :] if nch == ECH
                            else xT[:, c, j * P:(j + 1) * P],
                            w_sb[:, c, :],
                            start=(c == 0),
                            stop=(c == nch - 1 and brow is None),
                        )
                    if brow is not None:
                        nc.tensor.matmul(
                            ps[:, 0:E], ones_sb[:], brow[:], start=False, stop=True,
                        )
                    nc.vector.scalar_tensor_tensor(
                        out=h[i0 + j][:], in0=ps[:, 0:E], scalar=0.0,
                        in1=h[i0 + j][:], op0=Alu.add, op1=Alu.add,
                        accum_out=(None if sv is None else sv[:, i0 + j, 0:1]),
                    )

            def load_w(name, shape, dtp, tag=None):
                t = wpool.tile(shape, dtp, tag=tag or name[:-1])
                nc.sync.dma_start(out=t[:], in_=D[name].ap())
                return t

            scale = float(HS) ** -0.5
            # probs head -> physical slot (pairs share PSUM banks, see scores)
            SLOT = {0: 0, 2: 1, 4: 2, 1: 3, 3: 4, 5: 5}

            # ---- transformer layers ----
            for l in range(L):
                wq = load_w(f"wq{l}", [P, ECH, E], bf)
                wk = load_w(f"wk{l}", [P, ECH, E], bf)
                wv = load_w(f"wv{l}", [P, ECH, E], bf)
                wproj = load_w(f"wproj{l}", [P, ECH, E], bf)
                w1 = load_w(f"w1{l}", [P, ECH, FF], bf)
                w2 = load_w(f"w2{l}", [P, FCH, E], bf)
                bq = load_w(f"bq{l}", [P, ECH], f32) if bq_nz[l] else None
                bk = load_w(f"bk{l}", [P, ECH], f32) if bk_nz[l] else None
                bvrow = load_w(f"bvrow{l}", [1, E], bf) if bv_nz[l] else None
                bprow = load_w(f"bprow{l}", [1, E], bf) if bp_nz[l] else None
                b1c = load_w(f"b1c{l}", [P, FCH], f32) if b1_nz[l] else None
                b2row = load_w(f"b2row{l}", [1, E], bf) if b2_nz[l] else None

                # ======== attention pass ========
                m1, rstd1 = finish_stats(sv1, f"a{l}")
                sv2 = new_stats(f"m{l}")
                pend = None   # deferred proj of the previous block
                for b in range(NB):
                    i0 = 4 * b
                    xnT = make_xnT(i0, m1, rstd1)
                    QT = lin_fmaj(xnT, wq, bq, ECH, "QT", "dve")
                    KT = lin_fmaj(xnT, wk, bk, ECH, "KT", "act")
                    # V token-major, ones-augmented: [P, 4, H, 65]
                    Vt = act.tile([P, 4, H, 65], bf, tag="Vt")
                    for j in range(4):
                        ps = ps_lin.tile([P, 512], f32, tag="mm")
                        for c in range(ECH):
                            nc.tensor.matmul(
                                ps[:, 0:E], xnT[:, j * ECH + c, :], wv[:, c, :],
                                start=(c == 0),
                                stop=(c == ECH - 1 and bvrow is None),
                            )
                        if bvrow is not None:
                            nc.tensor.matmul(
                                ps[:, 0:E], ones_sb[:], bvrow[:],
                                start=False, stop=True,
                            )
                        nc.vector.tensor_copy(
                            out=Vt[:, j, :, 0:64],
                            in_=ps[:, 0:E].rearrange("p (h d) -> p h d", h=H),
                        )
                        nc.vector.memset(Vt[:, j, :, 64:65], 1.0)

                    # scores + exp + mask for BOTH sequences first (dense PE
                    # stream; exp latency hidden by the deferred proj below).
                    # scores are packed 2 heads (st=0) / 4 heads (st=1) per
                    # PSUM bank so each exp is one wide ACT instruction.
                    probs2 = []
                    for s in range(2):
                        tb = s * 256
                        probs = act2.tile([P, 2, H, 256], bf, tag="probs")
                        def score_mm(sc_reg, hh, st):
                            tlo = 128 if st == 1 else 0
                            c, off = divmod(hh * HS, P)
                            nc.tensor.matmul(
                                sc_reg,
                                KT[off:off + HS, c, tb + st * P: tb + (st + 1) * P],
                                QT[off:off + HS, c, tb + tlo: tb + 256],
                                start=True, stop=True,
                            )
                        # Heads sharing a PE row-group (even: rows 0-63, odd:
                        # rows 64-127) may drain a shared bank safely (strict
                        # FIFO); cross-group pairs would race on the bank's
                        # write port. Pack (0,2) and (1,3); 4 and 5 stay solo.
                        # probs stores heads in SLOT order so each pair's exp
                        # writes one contiguous region.
                        for st in range(2):
                            tlo = 128 if st == 1 else 0
                            w = 256 - tlo
                            for grp, sl0 in (((0, 2), 0), ((4,), 2),
                                             ((1, 3), 3), ((5,), 5)):
                                sc = ps_att.tile([P, 512], f32, tag="att", name="sc")
                                for k, hh in enumerate(grp):
                                    score_mm(sc[:, k * w:(k + 1) * w], hh, st)
                                nc.scalar.activation(
                                    out=probs[:, st, sl0:sl0 + len(grp), tlo:256],
                                    in_=sc[:, 0:len(grp) * w]
                                    .rearrange("p (k c) -> p k c", c=w),
                                    func=Act.Exp, scale=scale,
                                )
                        # causal mask: only the diagonal 128x128 needs it
                        for st in range(2):
                            tlo = 128 if st == 1 else 0
                            nc.vector.tensor_tensor(
                                out=probs[:, st, :, tlo:tlo + P],
                                in0=probs[:, st, :, tlo:tlo + P],
                                in1=mask_sb[:, None, :].to_broadcast((P, H, P)),
                                op=Alu.mult,
                            )
                        probs2.append(probs)

                    # deferred proj of block b-1 fills the PE while exp runs
                    if PIPE and pend is not None:
                        lin_tmaj_resid(pend[1], wproj, ECH, bprow, pend[0], sv2)
                        emit_stats(sv2, pend[0])

                    # o matmuls packed 4+2 heads per PSUM bank (all K=128,
                    # strict-FIFO drains -> no same-bank write races); softmax
                    # divide is one reciprocal + one broadcast TT per bank.
                    onorm4 = act.tile([P, 4, E], bf, tag="onorm4")
                    for s in range(2):
                        probs = probs2[s]
                        for tt in range(2):  # query tiles of this seq
                            for hg, nh in ((0, 4), (4, 2)):
                                po = ps_att.tile([P, 512], f32, tag="att", name="po")
                                for k in range(nh):
                                    hh = hg + k
                                    sl = SLOT[hh]
                                    dst = po[:, k * P:k * P + 65]
                                    if tt == 0:
                                        nc.tensor.matmul(
                                            dst, probs[:, 0, sl, 0:P],
                                            Vt[:, 2 * s, hh, :],
                                            start=True, stop=True,
                                        )
                                    else:
                                        nc.tensor.matmul(
                                            dst, probs[:, 0, sl, P:256],
                                            Vt[:, 2 * s, hh, :],
                                            start=True, stop=False,
                                        )
                                        nc.tensor.matmul(
                                            dst, probs[:, 1, sl, P:256],
                                            Vt[:, 2 * s + 1, hh, :],
                                            start=False, stop=True,
                                        )
                                pv = po[:].rearrange("p (k c) -> p k c", c=P)
                                rec = acts.tile([P, 4], f32, tag="rec")
                                nc.vector.reciprocal(
                                    out=rec[:, 0:nh], in_=pv[:, 0:nh, 64:65])
                                nc.vector.tensor_tensor(
                                    out=onorm4[:, 2 * s + tt, hg * 64:(hg + nh) * 64]
                                    .rearrange("p (k d) -> p k d", d=64),
                                    in0=pv[:, 0:nh, 0:64],
                                    in1=rec[:, 0:nh, None].to_broadcast((P, nh, 64)),
                                    op=Alu.mult,
                                )
                    oT = act.tile([P, 12, P], bf, tag="oT")
                    nc.sync.dma_start_transpose(
                        oT[:], onorm4[:].rearrange("p a b -> p (a b)"))
                    if PIPE:
                        pend = (i0, oT)
                    else:
                        lin_tmaj_resid(oT, wproj, ECH, bprow, i0, sv2)
                        emit_stats(sv2, i0)
                if PIPE:
                    lin_tmaj_resid(pend[1], wproj, ECH, bprow, pend[0], sv2)
                    emit_stats(sv2, pend[0])

                # ======== MLP pass ========
                m2, rstd2 = finish_stats(sv2, f"m{l}")
                sv1 = new_stats("f" if l == L - 1 else f"a{l + 1}")
                pend = None   # deferred W2 of the previous block
                for b in range(NB):
                    i0 = 4 * b
                    xnT = make_xnT(i0, m2, rstd2)
                    aT = lin_fmaj_relu(xnT, w1, b1c, "aT")
                    if PIPE and pend is not None:
                        lin_tmaj_resid(pend[1], w2, FCH, b2row, pend[0], sv1)
                        emit_stats(sv1, pend[0])
                    if PIPE:
                        pend = (i0, aT)
                    else:
                        lin_tmaj_resid(aT, w2, FCH, b2row, i0, sv1)
                        emit_stats(sv1, i0)
                if PIPE:
                    lin_tmaj_resid(pend[1], w2, FCH, b2row, pend[0], sv1)
                    emit_stats(sv1, pend[0])

            # ---- final LN + unembed (feature-major logits) ----
            wout = wpool.tile([P, ECH, V], bf, tag="wout")
            nc.sync.dma_start(out=wout[:], in_=D["wout"].ap())
            mf, rstdf = finish_stats(sv1, "f")
            for b in range(NB):
                xnT = make_xnT(4 * b, mf, rstdf)
                rhs_view = xnT[:].rearrange("p (j c) a -> p c j a", c=ECH)
                ps = ps_lin.tile([V, 512], f32, tag="mm", name="mmv")
                for c in range(ECH):
                    nc.tensor.matmul(
                        ps[:], wout[:, c, :], rhs_view[:, c],
                        start=(c == 0), stop=(c == ECH - 1),
                    )
                lt = act2.tile([V, 512], f32, tag="lt")
                if boutc_sb is not None:
                    nc.vector.tensor_scalar_add(
                        out=lt[:], in0=ps[:], scalar1=boutc_sb[:])
                else:
                    nc.vector.tensor_copy(out=lt[:], in_=ps[:])
                nc.sync.dma_start(
                    out=D["logT"].ap()[:, b * 512:(b + 1) * 512], in_=lt[:],
                )

    nc.compile()
    return nc


def _prep_shared(inp):
    """Host-side weight prep: layout rearrangement + LN gamma/beta folding."""
    sh = {}

    def f32(x):
        return np.asarray(x, np.float32)

    sh["te"] = np.asarray(f32(inp["tok_emb"]), BF16)                      # [V,E]
    sh["pos"] = np.ascontiguousarray(
        f32(inp["pos_emb"]).reshape(2, P, E).transpose(1, 0, 2))          # [P,2,E]
    sh["mask"] = np.asarray(np.triu(np.ones((P, P), np.float32)), BF16)   # [P,P]

    def tile3(w, fdim):  # [E, fdim] -> [P, ECH, fdim]
        return np.ascontiguousarray(w.reshape(ECH, P, fdim).transpose(1, 0, 2))

    def col(b, nch):  # [nch*P] -> [P, nch]
        return np.ascontiguousarray(b.reshape(nch, P).T)

    bq_nz, bk_nz, bv_nz, bp_nz, b1_nz, b2_nz = [], [], [], [], [], []
    for l in range(L):
        g1, b1_ = f32(inp["ln1_g"][l]), f32(inp["ln1_b"][l])
        g2, b2_ = f32(inp["ln2_g"][l]), f32(inp["ln2_b"][l])
        wq = f32(inp["Wq"][l]).transpose(1, 0, 2).reshape(E, E)   # head-major cols
        wk = f32(inp["Wk"][l]).transpose(1, 0, 2).reshape(E, E)
        wv = f32(inp["Wv"][l]).transpose(1, 0, 2).reshape(E, E)
        sh[f"wq{l}"] = np.asarray(tile3(g1[:, None] * wq, E), BF16)
        sh[f"wk{l}"] = np.asarray(tile3(g1[:, None] * wk, E), BF16)
        sh[f"wv{l}"] = np.asarray(tile3(g1[:, None] * wv, E), BF16)
        bq = wq.T @ b1_
        bk = wk.T @ b1_
        bv = wv.T @ b1_
        bq_nz.append(bool(np.any(bq != 0)))
        bk_nz.append(bool(np.any(bk != 0)))
        bv_nz.append(bool(np.any(bv != 0)))
        if bq_nz[-1]:
            sh[f"bq{l}"] = col(bq, ECH)
        if bk_nz[-1]:
            sh[f"bk{l}"] = col(bk, ECH)
        if bv_nz[-1]:
            sh[f"bvrow{l}"] = np.asarray(bv[None, :], BF16)
        wp = f32(inp["Wproj"][l])
        sh[f"wproj{l}"] = np.asarray(tile3(wp, E), BF16)
        bp = f32(inp["bproj"][l])
        bp_nz.append(bool(np.any(bp != 0)))
        if bp_nz[-1]:
            sh[f"bprow{l}"] = np.asarray(bp[None, :], BF16)
        w1 = f32(inp["W1"][l])
        sh[f"w1{l}"] = np.asarray(tile3(g2[:, None] * w1, FF), BF16)
        b1ff = f32(inp["b1"][l]) + w1.T @ b2_
        b1_nz.append(bool(np.any(b1ff != 0)))
        if b1_nz[-1]:
            sh[f"b1c{l}"] = col(b1ff, FCH)
        w2 = f32(inp["W2"][l])
        sh[f"w2{l}"] = np.asarray(
            w2.reshape(FCH, P, E).transpose(1, 0, 2), BF16)
        b2r = f32(inp["b2"][l])
        b2_nz.append(bool(np.any(b2r != 0)))
        if b2_nz[-1]:
            sh[f"b2row{l}"] = np.asarray(b2r[None, :], BF16)

    gf, bf_ = f32(inp["lnf_g"]), f32(inp["lnf_b"])
    wo = f32(inp["Wout"])
    sh["wout"] = np.asarray(tile3(gf[:, None] * wo, V), BF16)
    boutc = f32(inp["bout"]) + wo.T @ bf_
    bout_nz = bool(np.any(boutc != 0))
    if bout_nz:
        sh["boutc"] = boutc.reshape(V, 1)
    flags = (tuple(bq_nz), tuple(bk_nz), tuple(bv_nz), tuple(bp_nz),
             tuple(b1_nz), tuple(b2_nz), bout_nz)
    return sh, flags


def _onehot(xc):
    """xc: [BPC, T] ints -> [P, TOK] bf16 one-hot (feature-major, zero-padded
    to 128 rows so the embedding contraction uses a full partition dim)."""
    xf = np.asarray(xc, np.int64).reshape(-1)
    oh = np.zeros((P, TOK), np.float32)
    oh[xf, np.arange(TOK)] = 1.0
    return np.asarray(oh, BF16)


def _get_nc(flags):
    if flags not in _NC_CACHE:
        _NC_CACHE[flags] = _build_nc(flags)
    return _NC_CACHE[flags]


def make_in_maps(inputs):
    sh, flags = _prep_shared(inputs)
    x = np.asarray(inputs["x"])
    in_maps = []
    for c in range(NCORES):
        m = dict(sh)
        m["oh"] = _onehot(x[c * BPC:(c + 1) * BPC])
        in_maps.append(m)
    return in_maps, flags


def kernel(**inputs):
    import os
    from concourse.bass_utils import run_bass_kernel_spmd

    in_maps, flags = make_in_maps(inputs)
    nc = _get_nc(flags)
    kw = {}
    if os.environ.get("BASS_TRACE"):
        d = os.environ.get("BASS_TRACE_DIR", "/tmp/bass_trace")
        os.makedirs(d, exist_ok=True)
        kw["tmpdir"] = d
    res = run_bass_kernel_spmd(nc, in_maps, list(range(NCORES)), **kw)
    kernel._last = res
    outs = []
    for c in range(NCORES):
        lt = np.asarray(res.results[c]["logT"], np.float32)   # [V, TOK]
        outs.append(np.ascontiguousarray(lt.T).reshape(BPC, T, V))
    return np.concatenate(outs, axis=0)


kernel._last = None
